# revision 60
# baseline (speedup 1.0000x reference)
"""Trainium2 Bass kernel for nn_EncoderBlock (dense transformer encoder block).

Strategy: pure data parallelism — batch B=8 across the 8 NeuronCores, one
batch element per core. No collectives. Per core:

  LN1 -> q = n@wqT+bq (kh=vh=qh, reproducing the reference's q-reuse bug)
  per head: S = qh^T qh / 8 (symmetric), E = exp(S/8 - 20), Z row-sums via
  activation accum_out (symmetry of S), ctx = E-weighted sum of qh, normalized
  by 1/Z broadcast via DRAM-bounce DMA; wo projection + residual; LN2; ReLU
  FFN (d_ff=4096) streamed from HBM; residual; out.

Matmuls run in bf16 (fp32 accumulation in PSUM); layernorm statistics,
softmax sums and the residual stream stay fp32.
"""

import sys

sys.path.insert(0, "/opt/trn_rl_repo")

import numpy as np
import ml_dtypes
from contextlib import ExitStack

import concourse.bass as bass
import concourse.tile as tile
from concourse import bacc, mybir
from concourse import bass_utils
from concourse.bass import ts, ds
from concourse.masks import make_identity

BF = mybir.dt.bfloat16
F32 = mybir.dt.float32
AF = mybir.ActivationFunctionType
OP = mybir.AluOpType
AX = mybir.AxisListType

P = 128
S = 1024          # sequence length per core
D = 1024          # d_model
H = 16            # heads
DK = 64           # head dim
DFF = 4096
NB = 8            # batch = number of cores
SC = S // P       # 8 sequence chunks
DC = D // P       # 8 feature chunks
FC = DFF // P     # 32 ff chunks
EPS = 1e-6
EXP_SHIFT = -20.0  # constant shift inside exp; cancels in softmax ratio

last_exec_time_ns = None


def _emit_layernorm(nc, small, xt, n_out, alpha, beta, idx, chunks,
                    apply_eng=None):
    """LN with Bessel-corrected std (ddof=1), matching torch/jax reference:
    n = (x - mu)/(std + eps)*alpha + beta.  xt [P,*,D] f32 indexed by `chunks`,
    n_out [P,len(chunks),D] bf16 indexed locally.
    Stats per token live on partitions; mean+var via one bn_stats pass (DVE
    only — keeps ScalarE free for the softmax exps)."""
    chunks = list(chunks)
    nch = len(chunks)
    BSD = nc.vector.BN_STATS_DIM
    bsf = 512  # BN_STATS_FMAX; D = 2 subgroups of 512
    nsub = D // bsf
    bst = small.tile([P, nch, nsub, BSD], F32, name=f"ln{idx}_bst")
    mv = small.tile([P, nch, 2], F32, name=f"ln{idx}_mv")
    var = small.tile([P, nch], F32, name=f"ln{idx}_var")
    tmp = small.tile([P, nch], F32, name=f"ln{idx}_tmp")
    tcoef = small.tile([P, nch], F32, name=f"ln{idx}_t")
    ucoef = small.tile([P, nch], F32, name=f"ln{idx}_u")

    for i, sc in enumerate(chunks):
        xv = xt[:, sc].rearrange("p (ns f) -> p ns f", ns=nsub)
        for sg in range(nsub):
            nc.vector.bn_stats(bst[:, i, sg], xv[:, sg])
        nc.vector.bn_aggr(mv[:, i], bst[:, i])
    mu = mv[:, :, 0]  # strided [P, nch] views
    # var (ddof=1)
    nc.vector.tensor_scalar_mul(var[:], mv[:, :, 1], float(D) / (D - 1))
    # 1/(std+eps) ~= rsqrt(var) (eps=1e-6 << std~1, relative error ~1e-6),
    # computed entirely on the DVE: LN variances concentrate near 1, so a
    # Taylor seed sqrt(r)~=1+(r-1)/2-(r-1)^2/8 off r=1/var (approx_fast)
    # plus two multiply-only Newton steps y*(1.5-0.5*var*y^2) reaches
    # ~1e-6 rel.  No ScalarE Sqrt -> no ACT-table switches anywhere.
    r = small.tile([P, nch], F32, name=f"ln{idx}_r")
    t2 = small.tile([P, nch], F32, name=f"ln{idx}_t2")
    y = small.tile([P, nch], F32, name=f"ln{idx}_y")
    nc.vector.reciprocal_approx_fast(r[:], var[:])
    nc.vector.tensor_scalar(tmp[:], r[:], 1.0, None, OP.subtract)  # t=r-1
    nc.vector.tensor_mul(t2[:], tmp[:], tmp[:])
    nc.vector.tensor_scalar(tmp[:], tmp[:], 0.5, 1.0, OP.mult, OP.add)
    nc.vector.scalar_tensor_tensor(y[:], t2[:], -0.125, tmp[:],
                                   OP.mult, OP.add)                # y0
    for _ in range(2):
        nc.vector.tensor_mul(t2[:], y[:], y[:])
        nc.vector.tensor_mul(t2[:], t2[:], var[:])
        nc.vector.tensor_scalar(t2[:], t2[:], -0.5, 1.5, OP.mult, OP.add)
        nc.vector.tensor_mul(y[:], y[:], t2[:])
    nc.vector.tensor_scalar_mul(tcoef[:], y[:], float(alpha))
    nc.vector.tensor_mul(tmp[:], mu, tcoef[:])
    nc.vector.tensor_scalar(ucoef[:], tmp[:], -1.0, float(beta), OP.mult, OP.add)
    apply_eng = apply_eng or nc.vector
    for i, sc in enumerate(chunks):
        apply_eng.tensor_scalar(
            n_out[:, i], xt[:, sc], tcoef[:, ds(i, 1)], ucoef[:, ds(i, 1)],
            OP.mult, OP.add,
        )


def _emit_transpose(nc, pool, dst, src, ident, ca_range=range(8),
                    copy_engs=None):
    """dst = 8x8 block transpose of src; both [P, 8, 1024] (bf16).
    PSUM evacuations rotate over copy_engs (default Vector/Scalar split)."""
    for ca in ca_range:
        for cb in range(8):
            pt = pool.tile([P, P], src.dtype, tag="tp", bufs=4, name="tp")
            nc.tensor.transpose(pt[:], src[:, ca, ts(cb, P)], ident[:])
            engs = copy_engs or [nc.vector, nc.scalar]
            eng = engs[cb % len(engs)]
            if eng is nc.scalar:
                eng.copy(dst[:, cb, ts(ca, P)], pt[:])
            else:
                eng.tensor_copy(dst[:, cb, ts(ca, P)], pt[:])


def build_program(ln1a, ln1b, ln2a, ln2b, mask_all_ones):
    import os
    phase_stop = int(os.environ.get("BASSK_PHASE", "9"))
    nc = bacc.Bacc("TRN2", target_bir_lowering=False, debug=False)

    x_d = nc.dram_tensor("x", (S, D), F32, kind="ExternalInput").ap()
    wqT_d = nc.dram_tensor("wqT", (D, D), BF, kind="ExternalInput").ap()
    woT_d = nc.dram_tensor("woT", (D, D), BF, kind="ExternalInput").ap()
    w1T_d = nc.dram_tensor("w1T", (D, DFF), BF, kind="ExternalInput").ap()
    w2T_d = nc.dram_tensor("w2T", (DFF, D), BF, kind="ExternalInput").ap()
    bq_d = nc.dram_tensor("bq_v", (P, DC), F32, kind="ExternalInput").ap()
    b1_d = nc.dram_tensor("b1_v", (P, FC), F32, kind="ExternalInput").ap()
    bo_d = nc.dram_tensor("bo_rep", (P, D), BF, kind="ExternalInput").ap()
    b2_d = nc.dram_tensor("b2_rep", (P, D), BF, kind="ExternalInput").ap()
    if not mask_all_ones:
        m01_d = nc.dram_tensor("m01_v", (P, SC), F32, kind="ExternalInput").ap()
    out_d = nc.dram_tensor("out", (S, D), F32, kind="ExternalOutput").ap()

    x_r = x_d.rearrange("(sc p) d -> sc p d", p=P)
    wqT_r = wqT_d.rearrange("(kc p) o -> kc p o", p=P)
    woT_r = woT_d.rearrange("(oc p) d -> oc p d", p=P)
    w1_batched = w1T_d.rearrange("(dc p) f -> p dc f", p=P)
    w2_batched = w2T_d.rearrange("(fc p) d -> p fc d", p=P)
    out_r = out_d.rearrange("(sc p) d -> sc p d", p=P)

    with tile.TileContext(nc) as tc, ExitStack() as st:
        arena = st.enter_context(tc.tile_pool(name="arena", bufs=1))
        small = st.enter_context(tc.tile_pool(name="small", bufs=1))

        # ---- constants ----
        ident_b = small.tile([P, P], BF, name="ident_b")
        make_identity(nc, ident_b[:])
        ones_b = small.tile([P, P], BF, name="ones_b")
        nc.gpsimd.memset(ones_b[:], 1.0)
        ebias = small.tile([P, 1], F32, name="ebias")
        nc.gpsimd.memset(ebias[:], EXP_SHIFT)
        bq_sb = small.tile([P, DC], F32, name="bq_sb")
        nc.sync.dma_start(bq_sb[:], bq_d)
        b1_sb = small.tile([P, FC], F32, name="b1_sb")
        nc.sync.dma_start(b1_sb[:], b1_d)
        bo_rep = small.tile([P, D], BF, name="bo_rep")
        nc.sync.dma_start(bo_rep[:], bo_d)
        b2_rep = small.tile([P, D], BF, name="b2_rep")
        nc.sync.dma_start(b2_rep[:], b2_d)
        if not mask_all_ones:
            m01_sb = small.tile([P, SC], F32, name="m01_sb")
            nc.sync.dma_start(m01_sb[:], m01_d)

        dma_engines = [nc.sync, nc.scalar, nc.gpsimd]
        # DMA issue queues that never carry softmax exps — weight streams
        # during the attention/FFN overlap must not block the Scalar queue
        # (DMA_DIRECT2D there waits on FFN semaphores ahead of exps).
        dma_quiet = [nc.sync, nc.gpsimd]

        # ---- phase A inputs ----
        # x chunks first; scalar carries only x1 so the LN1 Sqrt chain can
        # start the moment pair 0 lands; gpsimd stays clear for LN applies;
        # wq/woT follow on sync (needed later).
        xt = arena.tile([P, SC, D], F32, tag="xt_h1", name="xt")
        x_eng = [nc.sync, nc.scalar, nc.gpsimd, nc.sync,
                 nc.scalar, nc.gpsimd, nc.sync, nc.scalar]
        for sc in range(SC):
            x_eng[sc].dma_start(xt[:, sc], x_r[sc])
        qq = arena.tile([P, 2 * DC, S], BF, tag="qq_out", name="qq")
        qT = qq[:, 0:DC]        # [o%P, oc, s]
        qh = qq[:, DC:2 * DC]   # [s%P, sc, o]
        n1 = arena.tile([P, SC, D], BF, tag="n1_ctx", name="n1")
        n1T = arena.tile([P, DC, S], BF, tag="n1T_woT", name="n1T")
        wq_sb = arena.tile([P, DC, D], BF, tag="wq_res1", name="wq_sb")
        for kc in range(DC):
            [nc.scalar, nc.gpsimd][kc % 2].dma_start(wq_sb[:, kc], wqT_r[kc])

        # ================= phase A: LN1, q projection, transposes ============
        # LN1 split into halves so the first q-proj matmuls (which only read
        # n1T columns 0:512 = seq chunks 0-3) start while LN of chunks 4-7
        # still runs on the DVE.
        with tc.tile_pool(name="psA", bufs=1, space="PSUM") as psA:
            # HAM warm-up: dead transposes keep the PE clock-gate's busy
            # window active while LN1 runs, so phase A starts at 2.4 GHz
            for _ in range(72):
                wup = psA.tile([P, P], BF, tag="tp", bufs=4, name="wup")
                nc.tensor.transpose(wup[:], ident_b[:], ident_b[:])
            # LN1 in chunk pairs: stats on Vector, applies on GpSimd, and
            # the n1 -> n1T transposes as XBAR DMA-transposes on Sync (one
            # [128,1024] DMA per chunk writes the whole strided slice) —
            # three disjoint queues, nothing blocks the next pair's stats,
            # and the PE does no transpose work at all.
            for pi in range(SC // 2):
                pr = range(2 * pi, 2 * pi + 2)
                _emit_layernorm(nc, small, xt, n1[:, ds(2 * pi, 2)],
                                ln1a, ln1b, f"1p{pi}", pr,
                                apply_eng=nc.gpsimd)
                for ca in pr:
                    nc.sync.dma_start_transpose(
                        n1T[:, :, ts(ca, P)], n1[:, ca])

            def qproj_oc(pool, tag, bufs, b, oc):
                pbt = pool.tile([P, 512], F32, tag=tag, bufs=bufs,
                                name="qps")
                for kc in range(DC):
                    nc.tensor.matmul(
                        pbt[:], wq_sb[:, kc, ts(oc, P)],
                        n1T[:, kc, ds(512 * b, 512)],
                        start=(kc == 0), stop=(kc == DC - 1),
                    )
                nc.vector.tensor_scalar(
                    qT[:, oc, ds(512 * b, 512)], pbt[:],
                    bq_sb[:, ds(oc, 1)], None, OP.add,
                )

            for oc in range(DC):
                qproj_oc(psA, "qps", 4, 0, oc)
                # qh for seq chunks 0-3 only needs qT columns 0:512 (b=0):
                # one XBAR DMA-transpose per oc right after its bias-evac
                nc.sync.dma_start_transpose(
                    qh[:, 0:SC // 2, ts(oc, P)], qT[:, oc, ds(0, 512)])

        def mk_prolog(psB):
            def prolog(hp):
                # deferred phase-A work for feature chunk hp, filling the PE
                # under attention half 0's exp-bound loop: q-proj columns
                # 512:1024 plus one DMA-transpose for the dependent qh half.
                qproj_oc(psB, "mix", 2, 1, hp)
                nc.sync.dma_start_transpose(
                    qh[:, SC // 2:SC, ts(hp, P)], qT[:, hp, ds(512, 512)])
            return prolog

        if phase_stop <= 1:
            for sc in range(SC):
                dt_ = arena.tile([P, D], F32, tag="dump", bufs=2, name="dump")
                nc.vector.tensor_copy(dt_[:], qh[:, sc])
                nc.sync.dma_start(out_r[sc], dt_[:])
            nc.compile()
            return nc

        # persistent across the pipelined halves
        ctxT = arena.tile([P, DC, S], BF, tag="n1_ctx", name="ctxT")
        woT_sb = arena.tile([P, DC, D], BF, tag="n1T_woT", name="woT_sb")
        for oc in range(DC):
            nc.sync.dma_start(woT_sb[:, oc], woT_r[oc])
        res1 = arena.tile([P, SC, D], F32, tag="wq_res1", name="res1")
        out_sb = None

        # ============ attention / wo / LN2 / FFN pipelined by query halves ===
        #
        # exp (ScalarE) is the serial bottleneck of attention; splitting all
        # loops over queries lets FFN(half 0) matmuls run under the exps of
        # attention(half 1).  Z = column sums of E come from ones-matmuls
        # (exact, mask-friendly), replicated across psum partitions so the
        # 1/Z normalization is a plain tensor_tensor against the ctx psum.

        def attn_half(psT, half, prolog=None):
            # Software-pipelined: scores(i+1) matmuls are EMITTED before
            # Z/ctx(i) so the in-order PE queue runs them underneath exp(i).
            # scp is double-buffered (2x2 banks); exp(i-1) must be done
            # reading before scores(i+1) write the same buffer.
            # `prolog(hp)` lets deferred phase-A work (q-proj b1, qh
            # transposes) fill the PE underneath this ACT-bound loop.
            iters = [(hp, c) for hp in range(H // 2) for c in range(SC)]
            state = {}

            def emit_scores(hp, c):
                sp = psT.tile([P, 1024], F32, tag="scp", bufs=2, name="scp")
                for hl in range(2):
                    lo = hl * DK
                    nc.tensor.matmul(
                        sp[:, ds(hl * 512, 512)],
                        qT[ds(lo, DK), hp, ts(c, P)],
                        qT[ds(lo, DK), hp, ds(512 * half, 512)],
                        start=True, stop=True,
                        tile_position=(lo, 0),
                    )
                return sp

            sp_pend = emit_scores(*iters[0])
            for idx, (hp, c) in enumerate(iters):
                sp = sp_pend
                sp_pend = (emit_scores(*iters[idx + 1])
                           if idx + 1 < len(iters) else None)
                if hp not in state:
                    if prolog is not None:
                        prolog(hp)
                    state[hp] = (
                        psT.tile([P, 512], F32, tag="ctxp", bufs=1,
                                 name="ctxp"),
                        psT.tile([P, 512], F32, tag="zps", bufs=1,
                                 name="zps"),
                        arena.tile([P, SC, 2 * 512], BF, tag="EC", bufs=2,
                                   name="ec"),
                    )
                cp, zp, ec = state[hp]
                nc.scalar.activation(
                    ec[:, c], sp[:], AF.Exp, bias=ebias[:], scale=0.125,
                )
                if not mask_all_ones:
                    nc.vector.tensor_scalar_mul(
                        ec[:, c], ec[:, c], m01_sb[:, ds(c, 1)],
                    )
                for hl in range(2):
                    nc.tensor.matmul(
                        zp[ds(hl * DK, DK), :],
                        ones_b[:, ds(hl * DK, DK)],
                        ec[:, c, ds(hl * 512, 512)],
                        start=(c == 0), stop=(c == SC - 1),
                        tile_position=(0, hl * DK),
                        skip_group_check=True,
                    )
                    nc.tensor.matmul(
                        cp[ds(hl * DK, DK), :],
                        qh[:, c, ds(hp * P + hl * DK, DK)],
                        ec[:, c, ds(hl * 512, 512)],
                        start=(c == 0), stop=(c == SC - 1),
                        tile_position=(0, hl * DK),
                        skip_group_check=True,
                    )
                if c == SC - 1:
                    rz = arena.tile([P, 512], F32, tag="rzab", bufs=2,
                                    name="rz")
                    nc.vector.reciprocal_approx_fast(rz[:], zp[:])
                    nc.vector.tensor_mul(
                        ctxT[:, hp, ds(512 * half, 512)], cp[:], rz[:],
                    )
                    del state[hp]

        def wo_half(psW, half):
            for sl in range(SC // 2):
                sc = half * (SC // 2) + sl
                xre = arena.tile([P, D], F32, tag="xre", bufs=2, name="xre")
                nc.sync.dma_start(xre[:], x_r[sc])
                # precombine x + bo off the critical path so each wo PSUM
                # bank frees after a single add
                nc.vector.tensor_add(xre[:], xre[:], bo_rep[:])
                for dh in range(2):
                    wp = psW.tile([P, 512], F32, tag="mix", bufs=2,
                                  name="wops")
                    for oc in range(DC):
                        nc.tensor.matmul(
                            wp[:], ctxT[:, oc, ts(sc, P)],
                            woT_sb[:, oc, ds(512 * dh, 512)],
                            start=(oc == 0), stop=(oc == DC - 1),
                        )
                    nc.vector.tensor_add(
                        res1[:, sc, ds(512 * dh, 512)], wp[:],
                        xre[:, ds(512 * dh, 512)],
                    )

        def ln2_half(psB, half):
            n2h = arena.tile([P, SC // 2, D], BF, tag="n2h", bufs=1,
                             name="n2h")
            chunks = range(half * (SC // 2), (half + 1) * (SC // 2))
            _emit_layernorm(nc, small, res1, n2h, ln2a, ln2b, f"2h{half}",
                            chunks)
            n2Th = arena.tile([P, DC, 512], BF, tag="n2th", bufs=1,
                              name="n2Th")
            for ca in range(SC // 2):
                nc.sync.dma_start_transpose(n2Th[:, :, ts(ca, P)],
                                            n2h[:, ca])
            for sc in chunks:
                nc.vector.tensor_add(res1[:, sc], res1[:, sc], b2_rep[:])
            return n2Th

        def ffn1_half(psB, wsp, half, n2Th):
            h1 = arena.tile([P, FC, 512], BF, tag="xt_h1", name="h1")
            for fc in range(FC):
                wts = wsp.tile([P, DC, P], BF, tag="w1s", bufs=3, name="w1s")
                # weight stream on gpsimd+scalar: both provably free of
                # compute-gated work in the FFN windows, and two queues
                # hide the per-DMA completion latency
                [nc.gpsimd, nc.scalar][fc % 2].dma_start(
                    wts[:], w1_batched[:, :, ts(fc, P)])
                fp = psB.tile([P, 512], F32, tag="mix", bufs=2, name="f1ps")
                for dc in range(DC):
                    nc.tensor.matmul(
                        fp[:], wts[:, dc], n2Th[:, dc, :],
                        start=(dc == 0), stop=(dc == DC - 1),
                    )
                nc.vector.tensor_scalar(
                    h1[:, fc], fp[:], b1_sb[:, ds(fc, 1)], 0.0,
                    OP.add, OP.max,
                )
            return h1

        def ffn2_half(psF2, wsp, half, h1):
            nonlocal out_sb
            if out_sb is None:
                out_sb = arena.tile([P, SC, D], F32, tag="qq_out",
                                    name="out_sb")
            for dh in range(2):
                ops = [psF2.tile([P, 512], F32, tag="f2ps", bufs=6,
                                 name="f2ps") for _ in range(4)]
                for fc2 in range(FC // 2):
                    w2t = wsp.tile([P, 2, 512], BF, tag="w2s", bufs=3,
                                   name="w2s")
                    [nc.gpsimd, nc.scalar][fc2 % 2].dma_start(
                        w2t[:],
                        w2_batched[:, ds(2 * fc2, 2), ds(512 * dh, 512)])
                    for fi in range(2):
                        fc = 2 * fc2 + fi
                        for sl in range(4):
                            nc.tensor.matmul(
                                ops[sl][:], h1[:, fc, ts(sl, P)], w2t[:, fi],
                                start=(fc == 0), stop=(fc == FC - 1),
                            )
                for sl in range(4):
                    sc = half * 4 + sl
                    nc.vector.tensor_add(
                        out_sb[:, sc, ds(512 * dh, 512)], ops[sl][:],
                        res1[:, sc, ds(512 * dh, 512)],
                    )
                    nc.sync.dma_start(
                        out_r[sc][:, ds(512 * dh, 512)],
                        out_sb[:, sc, ds(512 * dh, 512)],
                    )

        with tc.tile_pool(name="psMix", bufs=1, space="PSUM") as psB, \
             tc.tile_pool(name="wstream", bufs=1) as wsp:
            with tc.tile_pool(name="psAtt", bufs=1, space="PSUM") as psT:
                attn_half(psT, 0, prolog=mk_prolog(psB))
                wo_half(psB, 0)
                n2Th0 = ln2_half(psB, 0)
                h10 = ffn1_half(psB, wsp, 0, n2Th0)
                attn_half(psT, 1)
                wo_half(psB, 1)
                n2Th1 = ln2_half(psB, 1)
            with tc.tile_pool(name="psF2", bufs=1, space="PSUM") as psF2:
                ffn2_half(psF2, wsp, 0, h10)
                h11 = ffn1_half(psB, wsp, 1, n2Th1)
                ffn2_half(psF2, wsp, 1, h11)

    nc.compile()
    return nc


def _prep_inputs(inputs):
    f32 = lambda a: np.ascontiguousarray(np.asarray(a, dtype=np.float32))
    bfT = lambda a: np.ascontiguousarray(
        np.asarray(a, dtype=np.float32).T.astype(ml_dtypes.bfloat16))
    x = f32(inputs["x"])                      # [B, S, D]
    mask = np.asarray(inputs["src_mask"])     # [B, 1, 1, S] int32
    wqT = bfT(inputs["wq"])                   # [D, D] (in, out)
    woT = bfT(inputs["wo"])
    w1T = bfT(inputs["w1"])                   # [D, DFF]
    w2T = bfT(inputs["w2"])                   # [DFF, D]
    bq_v = np.ascontiguousarray(f32(inputs["bq"]).reshape(DC, P).T)
    b1_v = np.ascontiguousarray(f32(inputs["b1"]).reshape(FC, P).T)
    bo_rep = np.ascontiguousarray(
        np.tile(f32(inputs["bo"]), (P, 1)).astype(ml_dtypes.bfloat16))
    b2_rep = np.ascontiguousarray(
        np.tile(f32(inputs["b2"]), (P, 1)).astype(ml_dtypes.bfloat16))
    scal = lambda k: float(np.asarray(inputs[k]).reshape(-1)[0])
    ln = (scal("ln1_a"), scal("ln1_b"), scal("ln2_a"), scal("ln2_b"))
    mask_all_ones = bool((mask != 0).all())

    shared = dict(wqT=wqT, woT=woT, w1T=w1T, w2T=w2T, bq_v=bq_v, b1_v=b1_v,
                  bo_rep=bo_rep, b2_rep=b2_rep)
    in_maps = []
    for b in range(NB):
        m = dict(shared)
        m["x"] = np.ascontiguousarray(x[b])
        if not mask_all_ones:
            m01 = (mask[b].reshape(S) != 0).astype(np.float32)
            m["m01_v"] = np.ascontiguousarray(m01.reshape(SC, P).T)
            m["m01_rep"] = np.ascontiguousarray(np.tile(m01, (P, 1)))
        in_maps.append(m)
    return in_maps, ln, mask_all_ones


last_nc = None
last_in_maps = None


def kernel(**inputs):
    global last_nc, last_in_maps
    in_maps, ln, mask_all_ones = _prep_inputs(inputs)
    nc = build_program(*ln, mask_all_ones)
    last_nc, last_in_maps = nc, in_maps
    res = bass_utils.run_bass_kernel_spmd(
        nc, in_maps, core_ids=list(range(NB)), trace=False,
    )
    out = np.stack([np.asarray(res.results[b]["out"]) for b in range(NB)])
    return out.astype(np.float32)



# revision 64
# speedup vs baseline: 1.0005x; 1.0005x over previous
"""Trainium2 Bass kernel for nn_EncoderBlock (dense transformer encoder block).

Strategy: pure data parallelism — batch B=8 across the 8 NeuronCores, one
batch element per core. No collectives. Per core:

  LN1 -> q = n@wqT+bq (kh=vh=qh, reproducing the reference's q-reuse bug)
  per head: S = qh^T qh / 8 (symmetric), E = exp(S/8 - 20), Z row-sums via
  activation accum_out (symmetry of S), ctx = E-weighted sum of qh, normalized
  by 1/Z broadcast via DRAM-bounce DMA; wo projection + residual; LN2; ReLU
  FFN (d_ff=4096) streamed from HBM; residual; out.

Matmuls run in bf16 (fp32 accumulation in PSUM); layernorm statistics,
softmax sums and the residual stream stay fp32.
"""

import sys

sys.path.insert(0, "/opt/trn_rl_repo")

import numpy as np
import ml_dtypes
from contextlib import ExitStack

import concourse.bass as bass
import concourse.tile as tile
from concourse import bacc, mybir
from concourse import bass_utils
from concourse.bass import ts, ds
from concourse.masks import make_identity

BF = mybir.dt.bfloat16
F32 = mybir.dt.float32
AF = mybir.ActivationFunctionType
OP = mybir.AluOpType
AX = mybir.AxisListType

P = 128
S = 1024          # sequence length per core
D = 1024          # d_model
H = 16            # heads
DK = 64           # head dim
DFF = 4096
NB = 8            # batch = number of cores
SC = S // P       # 8 sequence chunks
DC = D // P       # 8 feature chunks
FC = DFF // P     # 32 ff chunks
EPS = 1e-6
EXP_SHIFT = -20.0  # constant shift inside exp; cancels in softmax ratio

last_exec_time_ns = None


def _emit_layernorm(nc, small, xt, n_out, alpha, beta, idx, chunks,
                    apply_eng=None):
    """LN with Bessel-corrected std (ddof=1), matching torch/jax reference:
    n = (x - mu)/(std + eps)*alpha + beta.  xt [P,*,D] f32 indexed by `chunks`,
    n_out [P,len(chunks),D] bf16 indexed locally.
    Stats per token live on partitions; mean+var via one bn_stats pass (DVE
    only — keeps ScalarE free for the softmax exps)."""
    chunks = list(chunks)
    nch = len(chunks)
    BSD = nc.vector.BN_STATS_DIM
    bsf = 512  # BN_STATS_FMAX; D = 2 subgroups of 512
    nsub = D // bsf
    bst = small.tile([P, nch, nsub, BSD], F32, name=f"ln{idx}_bst")
    mv = small.tile([P, nch, 2], F32, name=f"ln{idx}_mv")
    var = small.tile([P, nch], F32, name=f"ln{idx}_var")
    tmp = small.tile([P, nch], F32, name=f"ln{idx}_tmp")
    tcoef = small.tile([P, nch], F32, name=f"ln{idx}_t")
    ucoef = small.tile([P, nch], F32, name=f"ln{idx}_u")

    for i, sc in enumerate(chunks):
        xv = xt[:, sc].rearrange("p (ns f) -> p ns f", ns=nsub)
        for sg in range(nsub):
            nc.vector.bn_stats(bst[:, i, sg], xv[:, sg])
        nc.vector.bn_aggr(mv[:, i], bst[:, i])
    mu = mv[:, :, 0]  # strided [P, nch] views
    # var (ddof=1)
    nc.vector.tensor_scalar_mul(var[:], mv[:, :, 1], float(D) / (D - 1))
    # 1/(std+eps) ~= rsqrt(var) (eps=1e-6 << std~1, relative error ~1e-6),
    # computed entirely on the DVE: LN variances concentrate near 1, so a
    # Taylor seed sqrt(r)~=1+(r-1)/2-(r-1)^2/8 off r=1/var (approx_fast)
    # plus two multiply-only Newton steps y*(1.5-0.5*var*y^2) reaches
    # ~1e-6 rel.  No ScalarE Sqrt -> no ACT-table switches anywhere.
    r = small.tile([P, nch], F32, name=f"ln{idx}_r")
    t2 = small.tile([P, nch], F32, name=f"ln{idx}_t2")
    y = small.tile([P, nch], F32, name=f"ln{idx}_y")
    nc.vector.reciprocal(r[:], var[:])
    nc.vector.tensor_scalar(tmp[:], r[:], 1.0, None, OP.subtract)  # t=r-1
    nc.vector.tensor_mul(t2[:], tmp[:], tmp[:])
    nc.vector.tensor_scalar(tmp[:], tmp[:], 0.5, 1.0, OP.mult, OP.add)
    nc.vector.scalar_tensor_tensor(y[:], t2[:], -0.125, tmp[:],
                                   OP.mult, OP.add)                # y0
    for _ in range(2):
        nc.vector.tensor_mul(t2[:], y[:], y[:])
        nc.vector.tensor_mul(t2[:], t2[:], var[:])
        nc.vector.tensor_scalar(t2[:], t2[:], -0.5, 1.5, OP.mult, OP.add)
        nc.vector.tensor_mul(y[:], y[:], t2[:])
    nc.vector.tensor_scalar_mul(tcoef[:], y[:], float(alpha))
    nc.vector.tensor_mul(tmp[:], mu, tcoef[:])
    nc.vector.tensor_scalar(ucoef[:], tmp[:], -1.0, float(beta), OP.mult, OP.add)
    apply_eng = apply_eng or nc.vector
    for i, sc in enumerate(chunks):
        apply_eng.tensor_scalar(
            n_out[:, i], xt[:, sc], tcoef[:, ds(i, 1)], ucoef[:, ds(i, 1)],
            OP.mult, OP.add,
        )


def _emit_transpose(nc, pool, dst, src, ident, ca_range=range(8),
                    copy_engs=None):
    """dst = 8x8 block transpose of src; both [P, 8, 1024] (bf16).
    PSUM evacuations rotate over copy_engs (default Vector/Scalar split)."""
    for ca in ca_range:
        for cb in range(8):
            pt = pool.tile([P, P], src.dtype, tag="tp", bufs=4, name="tp")
            nc.tensor.transpose(pt[:], src[:, ca, ts(cb, P)], ident[:])
            engs = copy_engs or [nc.vector, nc.scalar]
            eng = engs[cb % len(engs)]
            if eng is nc.scalar:
                eng.copy(dst[:, cb, ts(ca, P)], pt[:])
            else:
                eng.tensor_copy(dst[:, cb, ts(ca, P)], pt[:])


def build_program(ln1a, ln1b, ln2a, ln2b, mask_all_ones):
    import os
    phase_stop = int(os.environ.get("BASSK_PHASE", "9"))
    nc = bacc.Bacc("TRN2", target_bir_lowering=False, debug=False)

    x_d = nc.dram_tensor("x", (S, D), F32, kind="ExternalInput").ap()
    wqT_d = nc.dram_tensor("wqT", (D, D), BF, kind="ExternalInput").ap()
    woT_d = nc.dram_tensor("woT", (D, D), BF, kind="ExternalInput").ap()
    w1T_d = nc.dram_tensor("w1T", (D, DFF), BF, kind="ExternalInput").ap()
    w2T_d = nc.dram_tensor("w2T", (DFF, D), BF, kind="ExternalInput").ap()
    bq_d = nc.dram_tensor("bq_v", (P, DC), F32, kind="ExternalInput").ap()
    b1_d = nc.dram_tensor("b1_v", (P, FC), F32, kind="ExternalInput").ap()
    bo_d = nc.dram_tensor("bo_rep", (P, D), BF, kind="ExternalInput").ap()
    b2_d = nc.dram_tensor("b2_rep", (P, D), BF, kind="ExternalInput").ap()
    if not mask_all_ones:
        m01_d = nc.dram_tensor("m01_v", (P, SC), F32, kind="ExternalInput").ap()
    out_d = nc.dram_tensor("out", (S, D), F32, kind="ExternalOutput").ap()

    x_r = x_d.rearrange("(sc p) d -> sc p d", p=P)
    wqT_r = wqT_d.rearrange("(kc p) o -> kc p o", p=P)
    woT_r = woT_d.rearrange("(oc p) d -> oc p d", p=P)
    w1_batched = w1T_d.rearrange("(dc p) f -> p dc f", p=P)
    w2_batched = w2T_d.rearrange("(fc p) d -> p fc d", p=P)
    out_r = out_d.rearrange("(sc p) d -> sc p d", p=P)

    with tile.TileContext(nc) as tc, ExitStack() as st:
        arena = st.enter_context(tc.tile_pool(name="arena", bufs=1))
        small = st.enter_context(tc.tile_pool(name="small", bufs=1))

        # ---- constants ----
        ident_b = small.tile([P, P], BF, name="ident_b")
        make_identity(nc, ident_b[:])
        ones_b = small.tile([P, P], BF, name="ones_b")
        nc.gpsimd.memset(ones_b[:], 1.0)
        ebias = small.tile([P, 1], F32, name="ebias")
        nc.gpsimd.memset(ebias[:], EXP_SHIFT)
        bq_sb = small.tile([P, DC], F32, name="bq_sb")
        b1_sb = small.tile([P, FC], F32, name="b1_sb")
        bo_rep = small.tile([P, D], BF, name="bo_rep")
        b2_rep = small.tile([P, D], BF, name="b2_rep")
        if not mask_all_ones:
            m01_sb = small.tile([P, SC], F32, name="m01_sb")
            nc.sync.dma_start(m01_sb[:], m01_d)

        def emit_bias_dmas():
            # biases are needed late; keep their DMAs behind the x chunks
            nc.sync.dma_start(bq_sb[:], bq_d)
            nc.sync.dma_start(b1_sb[:], b1_d)
            nc.sync.dma_start(bo_rep[:], bo_d)
            nc.sync.dma_start(b2_rep[:], b2_d)

        dma_engines = [nc.sync, nc.scalar, nc.gpsimd]
        # DMA issue queues that never carry softmax exps — weight streams
        # during the attention/FFN overlap must not block the Scalar queue
        # (DMA_DIRECT2D there waits on FFN semaphores ahead of exps).
        dma_quiet = [nc.sync, nc.gpsimd]

        # ---- phase A inputs ----
        # x chunks first; scalar carries only x1 so the LN1 Sqrt chain can
        # start the moment pair 0 lands; gpsimd stays clear for LN applies;
        # wq/woT follow on sync (needed later).
        xt = arena.tile([P, SC, D], F32, tag="xt_h1", name="xt")
        x_eng = [nc.sync, nc.scalar, nc.gpsimd, nc.sync,
                 nc.scalar, nc.gpsimd, nc.sync, nc.scalar]
        for sc in range(SC):
            x_eng[sc].dma_start(xt[:, sc], x_r[sc])
        qq = arena.tile([P, 2 * DC, S], BF, tag="qq_out", name="qq")
        qT = qq[:, 0:DC]        # [o%P, oc, s]
        qh = qq[:, DC:2 * DC]   # [s%P, sc, o]
        n1 = arena.tile([P, SC, D], BF, tag="n1_ctx", name="n1")
        n1T = arena.tile([P, DC, S], BF, tag="n1T_woT", name="n1T")
        wq_sb = arena.tile([P, DC, D], BF, tag="wq_res1", name="wq_sb")
        for kc in range(DC):
            [nc.scalar, nc.gpsimd][kc % 2].dma_start(wq_sb[:, kc], wqT_r[kc])
        emit_bias_dmas()

        # ================= phase A: LN1, q projection, transposes ============
        # LN1 split into halves so the first q-proj matmuls (which only read
        # n1T columns 0:512 = seq chunks 0-3) start while LN of chunks 4-7
        # still runs on the DVE.
        with tc.tile_pool(name="psA", bufs=1, space="PSUM") as psA:
            # HAM warm-up: dead transposes keep the PE clock-gate's busy
            # window active while LN1 runs, so phase A starts at 2.4 GHz
            for _ in range(72):
                wup = psA.tile([P, P], BF, tag="tp", bufs=4, name="wup")
                nc.tensor.transpose(wup[:], ident_b[:], ident_b[:])
            # LN1 in chunk pairs: stats on Vector, applies on GpSimd, and
            # the n1 -> n1T transposes as XBAR DMA-transposes on Sync (one
            # [128,1024] DMA per chunk writes the whole strided slice) —
            # three disjoint queues, nothing blocks the next pair's stats,
            # and the PE does no transpose work at all.
            for pi in range(SC // 2):
                pr = range(2 * pi, 2 * pi + 2)
                _emit_layernorm(nc, small, xt, n1[:, ds(2 * pi, 2)],
                                ln1a, ln1b, f"1p{pi}", pr,
                                apply_eng=nc.gpsimd)
                for ca in pr:
                    nc.sync.dma_start_transpose(
                        n1T[:, :, ts(ca, P)], n1[:, ca])

            def qproj_oc(pool, tag, bufs, b, oc):
                pbt = pool.tile([P, 512], F32, tag=tag, bufs=bufs,
                                name="qps")
                for kc in range(DC):
                    nc.tensor.matmul(
                        pbt[:], wq_sb[:, kc, ts(oc, P)],
                        n1T[:, kc, ds(512 * b, 512)],
                        start=(kc == 0), stop=(kc == DC - 1),
                    )
                nc.vector.tensor_scalar(
                    qT[:, oc, ds(512 * b, 512)], pbt[:],
                    bq_sb[:, ds(oc, 1)], None, OP.add,
                )

            for oc in range(DC):
                qproj_oc(psA, "qps", 4, 0, oc)
                # qh for seq chunks 0-3 only needs qT columns 0:512 (b=0):
                # one XBAR DMA-transpose per oc right after its bias-evac
                nc.sync.dma_start_transpose(
                    qh[:, 0:SC // 2, ts(oc, P)], qT[:, oc, ds(0, 512)])

        def mk_prolog(psB):
            def prolog(hp):
                # deferred phase-A work for feature chunk hp, filling the PE
                # under attention half 0's exp-bound loop: q-proj columns
                # 512:1024 plus one DMA-transpose for the dependent qh half.
                qproj_oc(psB, "mix", 2, 1, hp)
                nc.sync.dma_start_transpose(
                    qh[:, SC // 2:SC, ts(hp, P)], qT[:, hp, ds(512, 512)])
            return prolog

        if phase_stop <= 1:
            for sc in range(SC):
                dt_ = arena.tile([P, D], F32, tag="dump", bufs=2, name="dump")
                nc.vector.tensor_copy(dt_[:], qh[:, sc])
                nc.sync.dma_start(out_r[sc], dt_[:])
            nc.compile()
            return nc

        # persistent across the pipelined halves
        ctxT = arena.tile([P, DC, S], BF, tag="n1_ctx", name="ctxT")
        woT_sb = arena.tile([P, DC, D], BF, tag="n1T_woT", name="woT_sb")
        for oc in range(DC):
            nc.sync.dma_start(woT_sb[:, oc], woT_r[oc])
        res1 = arena.tile([P, SC, D], F32, tag="wq_res1", name="res1")
        out_sb = None

        # ============ attention / wo / LN2 / FFN pipelined by query halves ===
        #
        # exp (ScalarE) is the serial bottleneck of attention; splitting all
        # loops over queries lets FFN(half 0) matmuls run under the exps of
        # attention(half 1).  Z = column sums of E come from ones-matmuls
        # (exact, mask-friendly), replicated across psum partitions so the
        # 1/Z normalization is a plain tensor_tensor against the ctx psum.

        def attn_half(psT, half, prolog=None):
            # Software-pipelined: scores(i+1) matmuls are EMITTED before
            # Z/ctx(i) so the in-order PE queue runs them underneath exp(i).
            # scp is double-buffered (2x2 banks); exp(i-1) must be done
            # reading before scores(i+1) write the same buffer.
            # `prolog(hp)` lets deferred phase-A work (q-proj b1, qh
            # transposes) fill the PE underneath this ACT-bound loop.
            iters = [(hp, c) for hp in range(H // 2) for c in range(SC)]
            state = {}

            def emit_scores(hp, c):
                sp = psT.tile([P, 1024], F32, tag="scp", bufs=2, name="scp")
                for hl in range(2):
                    lo = hl * DK
                    nc.tensor.matmul(
                        sp[:, ds(hl * 512, 512)],
                        qT[ds(lo, DK), hp, ts(c, P)],
                        qT[ds(lo, DK), hp, ds(512 * half, 512)],
                        start=True, stop=True,
                        tile_position=(lo, 0),
                    )
                return sp

            sp_pend = emit_scores(*iters[0])
            for idx, (hp, c) in enumerate(iters):
                sp = sp_pend
                sp_pend = (emit_scores(*iters[idx + 1])
                           if idx + 1 < len(iters) else None)
                if hp not in state:
                    if prolog is not None:
                        prolog(hp)
                    state[hp] = (
                        psT.tile([P, 512], F32, tag="ctxp", bufs=1,
                                 name="ctxp"),
                        psT.tile([P, 512], F32, tag="zps", bufs=1,
                                 name="zps"),
                        arena.tile([P, SC, 2 * 512], BF, tag="EC", bufs=2,
                                   name="ec"),
                    )
                cp, zp, ec = state[hp]
                nc.scalar.activation(
                    ec[:, c], sp[:], AF.Exp, bias=ebias[:], scale=0.125,
                )
                if not mask_all_ones:
                    nc.vector.tensor_scalar_mul(
                        ec[:, c], ec[:, c], m01_sb[:, ds(c, 1)],
                    )
                for hl in range(2):
                    nc.tensor.matmul(
                        zp[ds(hl * DK, DK), :],
                        ones_b[:, ds(hl * DK, DK)],
                        ec[:, c, ds(hl * 512, 512)],
                        start=(c == 0), stop=(c == SC - 1),
                        tile_position=(0, hl * DK),
                        skip_group_check=True,
                    )
                    nc.tensor.matmul(
                        cp[ds(hl * DK, DK), :],
                        qh[:, c, ds(hp * P + hl * DK, DK)],
                        ec[:, c, ds(hl * 512, 512)],
                        start=(c == 0), stop=(c == SC - 1),
                        tile_position=(0, hl * DK),
                        skip_group_check=True,
                    )
                if c == SC - 1:
                    rz = arena.tile([P, 512], F32, tag="rzab", bufs=2,
                                    name="rz")
                    nc.vector.reciprocal_approx_fast(rz[:], zp[:])
                    nc.vector.tensor_mul(
                        ctxT[:, hp, ds(512 * half, 512)], cp[:], rz[:],
                    )
                    del state[hp]

        def wo_half(psW, half):
            for sl in range(SC // 2):
                sc = half * (SC // 2) + sl
                xre = arena.tile([P, D], F32, tag="xre", bufs=2, name="xre")
                nc.sync.dma_start(xre[:], x_r[sc])
                # precombine x + bo off the critical path so each wo PSUM
                # bank frees after a single add
                nc.vector.tensor_add(xre[:], xre[:], bo_rep[:])
                for dh in range(2):
                    wp = psW.tile([P, 512], F32, tag="mix", bufs=2,
                                  name="wops")
                    for oc in range(DC):
                        nc.tensor.matmul(
                            wp[:], ctxT[:, oc, ts(sc, P)],
                            woT_sb[:, oc, ds(512 * dh, 512)],
                            start=(oc == 0), stop=(oc == DC - 1),
                        )
                    nc.vector.tensor_add(
                        res1[:, sc, ds(512 * dh, 512)], wp[:],
                        xre[:, ds(512 * dh, 512)],
                    )

        def ln2_half(psB, half):
            n2h = arena.tile([P, SC // 2, D], BF, tag="n2h", bufs=1,
                             name="n2h")
            chunks = range(half * (SC // 2), (half + 1) * (SC // 2))
            _emit_layernorm(nc, small, res1, n2h, ln2a, ln2b, f"2h{half}",
                            chunks)
            n2Th = arena.tile([P, DC, 512], BF, tag="n2th", bufs=1,
                              name="n2Th")
            for ca in range(SC // 2):
                nc.sync.dma_start_transpose(n2Th[:, :, ts(ca, P)],
                                            n2h[:, ca])
            for sc in chunks:
                nc.vector.tensor_add(res1[:, sc], res1[:, sc], b2_rep[:])
            return n2Th

        def ffn1_half(psB, wsp, half, n2Th):
            h1 = arena.tile([P, FC, 512], BF, tag="xt_h1", name="h1")
            # two queues hide the per-DMA completion latency.  Half 0 runs
            # concurrently with attention-half-1 exps, so its second queue
            # is Sync (Scalar would stall exps behind buffer-gated DMAs);
            # half 1 runs when Scalar is exp-free.
            w1q = [nc.gpsimd, nc.sync if half == 0 else nc.scalar]
            for fc in range(FC):
                wts = wsp.tile([P, DC, P], BF, tag="w1s", bufs=3, name="w1s")
                w1q[fc % 2].dma_start(wts[:], w1_batched[:, :, ts(fc, P)])
                fp = psB.tile([P, 512], F32, tag="mix", bufs=2, name="f1ps")
                for dc in range(DC):
                    nc.tensor.matmul(
                        fp[:], wts[:, dc], n2Th[:, dc, :],
                        start=(dc == 0), stop=(dc == DC - 1),
                    )
                nc.vector.tensor_scalar(
                    h1[:, fc], fp[:], b1_sb[:, ds(fc, 1)], 0.0,
                    OP.add, OP.max,
                )
            return h1

        def ffn2_half(psF2, wsp, half, h1):
            nonlocal out_sb
            if out_sb is None:
                out_sb = arena.tile([P, SC, D], F32, tag="qq_out",
                                    name="out_sb")
            for dh in range(2):
                ops = [psF2.tile([P, 512], F32, tag="f2ps", bufs=6,
                                 name="f2ps") for _ in range(4)]
                for fc2 in range(FC // 2):
                    w2t = wsp.tile([P, 2, 512], BF, tag="w2s", bufs=3,
                                   name="w2s")
                    [nc.gpsimd, nc.scalar][fc2 % 2].dma_start(
                        w2t[:],
                        w2_batched[:, ds(2 * fc2, 2), ds(512 * dh, 512)])
                    for fi in range(2):
                        fc = 2 * fc2 + fi
                        for sl in range(4):
                            nc.tensor.matmul(
                                ops[sl][:], h1[:, fc, ts(sl, P)], w2t[:, fi],
                                start=(fc == 0), stop=(fc == FC - 1),
                            )
                for sl in range(4):
                    sc = half * 4 + sl
                    nc.vector.tensor_add(
                        out_sb[:, sc, ds(512 * dh, 512)], ops[sl][:],
                        res1[:, sc, ds(512 * dh, 512)],
                    )
                    nc.sync.dma_start(
                        out_r[sc][:, ds(512 * dh, 512)],
                        out_sb[:, sc, ds(512 * dh, 512)],
                    )

        with tc.tile_pool(name="psMix", bufs=1, space="PSUM") as psB, \
             tc.tile_pool(name="wstream", bufs=1) as wsp:
            with tc.tile_pool(name="psAtt", bufs=1, space="PSUM") as psT:
                attn_half(psT, 0, prolog=mk_prolog(psB))
                wo_half(psB, 0)
                n2Th0 = ln2_half(psB, 0)
                h10 = ffn1_half(psB, wsp, 0, n2Th0)
                attn_half(psT, 1)
                wo_half(psB, 1)
                n2Th1 = ln2_half(psB, 1)
            with tc.tile_pool(name="psF2", bufs=1, space="PSUM") as psF2:
                ffn2_half(psF2, wsp, 0, h10)
                h11 = ffn1_half(psB, wsp, 1, n2Th1)
                ffn2_half(psF2, wsp, 1, h11)

    nc.compile()
    return nc


def _prep_inputs(inputs):
    f32 = lambda a: np.ascontiguousarray(np.asarray(a, dtype=np.float32))
    bfT = lambda a: np.ascontiguousarray(
        np.asarray(a, dtype=np.float32).T.astype(ml_dtypes.bfloat16))
    x = f32(inputs["x"])                      # [B, S, D]
    mask = np.asarray(inputs["src_mask"])     # [B, 1, 1, S] int32
    wqT = bfT(inputs["wq"])                   # [D, D] (in, out)
    woT = bfT(inputs["wo"])
    w1T = bfT(inputs["w1"])                   # [D, DFF]
    w2T = bfT(inputs["w2"])                   # [DFF, D]
    bq_v = np.ascontiguousarray(f32(inputs["bq"]).reshape(DC, P).T)
    b1_v = np.ascontiguousarray(f32(inputs["b1"]).reshape(FC, P).T)
    bo_rep = np.ascontiguousarray(
        np.tile(f32(inputs["bo"]), (P, 1)).astype(ml_dtypes.bfloat16))
    b2_rep = np.ascontiguousarray(
        np.tile(f32(inputs["b2"]), (P, 1)).astype(ml_dtypes.bfloat16))
    scal = lambda k: float(np.asarray(inputs[k]).reshape(-1)[0])
    ln = (scal("ln1_a"), scal("ln1_b"), scal("ln2_a"), scal("ln2_b"))
    mask_all_ones = bool((mask != 0).all())

    shared = dict(wqT=wqT, woT=woT, w1T=w1T, w2T=w2T, bq_v=bq_v, b1_v=b1_v,
                  bo_rep=bo_rep, b2_rep=b2_rep)
    in_maps = []
    for b in range(NB):
        m = dict(shared)
        m["x"] = np.ascontiguousarray(x[b])
        if not mask_all_ones:
            m01 = (mask[b].reshape(S) != 0).astype(np.float32)
            m["m01_v"] = np.ascontiguousarray(m01.reshape(SC, P).T)
            m["m01_rep"] = np.ascontiguousarray(np.tile(m01, (P, 1)))
        in_maps.append(m)
    return in_maps, ln, mask_all_ones


last_nc = None
last_in_maps = None


def kernel(**inputs):
    global last_nc, last_in_maps
    in_maps, ln, mask_all_ones = _prep_inputs(inputs)
    nc = build_program(*ln, mask_all_ones)
    last_nc, last_in_maps = nc, in_maps
    res = bass_utils.run_bass_kernel_spmd(
        nc, in_maps, core_ids=list(range(NB)), trace=False,
    )
    out = np.stack([np.asarray(res.results[b]["out"]) for b in range(NB)])
    return out.astype(np.float32)



# revision 68
# speedup vs baseline: 1.0858x; 1.0853x over previous
"""Trainium2 Bass kernel for nn_EncoderBlock (dense transformer encoder block).

Strategy: pure data parallelism — batch B=8 across the 8 NeuronCores, one
batch element per core. No collectives. Per core:

  LN1 -> q = n@wqT+bq (kh=vh=qh, reproducing the reference's q-reuse bug)
  per head: S = qh^T qh / 8 (symmetric), E = exp(S/8 - 20), Z row-sums via
  activation accum_out (symmetry of S), ctx = E-weighted sum of qh, normalized
  by 1/Z broadcast via DRAM-bounce DMA; wo projection + residual; LN2; ReLU
  FFN (d_ff=4096) streamed from HBM; residual; out.

Matmuls run in bf16 (fp32 accumulation in PSUM); layernorm statistics,
softmax sums and the residual stream stay fp32.
"""

import sys

sys.path.insert(0, "/opt/trn_rl_repo")

import numpy as np
import ml_dtypes
from contextlib import ExitStack

import concourse.bass as bass
import concourse.tile as tile
from concourse import bacc, mybir
from concourse import bass_utils
from concourse.bass import ts, ds
from concourse.masks import make_identity

BF = mybir.dt.bfloat16
F32 = mybir.dt.float32
AF = mybir.ActivationFunctionType
OP = mybir.AluOpType
AX = mybir.AxisListType

P = 128
S = 1024          # sequence length per core
D = 1024          # d_model
H = 16            # heads
DK = 64           # head dim
DFF = 4096
NB = 8            # batch = number of cores
SC = S // P       # 8 sequence chunks
DC = D // P       # 8 feature chunks
FC = DFF // P     # 32 ff chunks
EPS = 1e-6
EXP_SHIFT = -20.0  # constant shift inside exp; cancels in softmax ratio

last_exec_time_ns = None


def _emit_layernorm(nc, small, xt, n_out, alpha, beta, idx, chunks,
                    apply_eng=None):
    """LN with Bessel-corrected std (ddof=1), matching torch/jax reference:
    n = (x - mu)/(std + eps)*alpha + beta.  xt [P,*,D] f32 indexed by `chunks`,
    n_out [P,len(chunks),D] bf16 indexed locally.
    Stats per token live on partitions; mean+var via one bn_stats pass (DVE
    only — keeps ScalarE free for the softmax exps)."""
    chunks = list(chunks)
    nch = len(chunks)
    BSD = nc.vector.BN_STATS_DIM
    bsf = 512  # BN_STATS_FMAX; D = 2 subgroups of 512
    nsub = D // bsf
    bst = small.tile([P, nch, nsub, BSD], F32, name=f"ln{idx}_bst")
    mv = small.tile([P, nch, 2], F32, name=f"ln{idx}_mv")
    var = small.tile([P, nch], F32, name=f"ln{idx}_var")
    tmp = small.tile([P, nch], F32, name=f"ln{idx}_tmp")
    tcoef = small.tile([P, nch], F32, name=f"ln{idx}_t")
    ucoef = small.tile([P, nch], F32, name=f"ln{idx}_u")

    for i, sc in enumerate(chunks):
        xv = xt[:, sc].rearrange("p (ns f) -> p ns f", ns=nsub)
        for sg in range(nsub):
            nc.vector.bn_stats(bst[:, i, sg], xv[:, sg])
        nc.vector.bn_aggr(mv[:, i], bst[:, i])
    mu = mv[:, :, 0]  # strided [P, nch] views
    # var (ddof=1)
    nc.vector.tensor_scalar_mul(var[:], mv[:, :, 1], float(D) / (D - 1))
    # 1/(std+eps) ~= rsqrt(var) (eps=1e-6 << std~1, relative error ~1e-6),
    # computed entirely on the DVE: LN variances concentrate near 1, so a
    # Taylor seed sqrt(r)~=1+(r-1)/2-(r-1)^2/8 off r=1/var (approx_fast)
    # plus two multiply-only Newton steps y*(1.5-0.5*var*y^2) reaches
    # ~1e-6 rel.  No ScalarE Sqrt -> no ACT-table switches anywhere.
    r = small.tile([P, nch], F32, name=f"ln{idx}_r")
    t2 = small.tile([P, nch], F32, name=f"ln{idx}_t2")
    y = small.tile([P, nch], F32, name=f"ln{idx}_y")
    nc.vector.reciprocal(r[:], var[:])
    nc.vector.tensor_scalar(tmp[:], r[:], 1.0, None, OP.subtract)  # t=r-1
    nc.vector.tensor_mul(t2[:], tmp[:], tmp[:])
    nc.vector.tensor_scalar(tmp[:], tmp[:], 0.5, 1.0, OP.mult, OP.add)
    nc.vector.scalar_tensor_tensor(y[:], t2[:], -0.125, tmp[:],
                                   OP.mult, OP.add)                # y0
    for _ in range(2):
        nc.vector.tensor_mul(t2[:], y[:], y[:])
        nc.vector.tensor_mul(t2[:], t2[:], var[:])
        nc.vector.tensor_scalar(t2[:], t2[:], -0.5, 1.5, OP.mult, OP.add)
        nc.vector.tensor_mul(y[:], y[:], t2[:])
    nc.vector.tensor_scalar_mul(tcoef[:], y[:], float(alpha))
    nc.vector.tensor_mul(tmp[:], mu, tcoef[:])
    nc.vector.tensor_scalar(ucoef[:], tmp[:], -1.0, float(beta), OP.mult, OP.add)
    apply_eng = apply_eng or nc.vector
    for i, sc in enumerate(chunks):
        apply_eng.tensor_scalar(
            n_out[:, i], xt[:, sc], tcoef[:, ds(i, 1)], ucoef[:, ds(i, 1)],
            OP.mult, OP.add,
        )


def _emit_transpose(nc, pool, dst, src, ident, ca_range=range(8),
                    copy_engs=None):
    """dst = 8x8 block transpose of src; both [P, 8, 1024] (bf16).
    PSUM evacuations rotate over copy_engs (default Vector/Scalar split)."""
    for ca in ca_range:
        for cb in range(8):
            pt = pool.tile([P, P], src.dtype, tag="tp", bufs=4, name="tp")
            nc.tensor.transpose(pt[:], src[:, ca, ts(cb, P)], ident[:])
            engs = copy_engs or [nc.vector, nc.scalar]
            eng = engs[cb % len(engs)]
            if eng is nc.scalar:
                eng.copy(dst[:, cb, ts(ca, P)], pt[:])
            else:
                eng.tensor_copy(dst[:, cb, ts(ca, P)], pt[:])


def build_program(ln1a, ln1b, ln2a, ln2b, mask_all_ones):
    import os
    phase_stop = int(os.environ.get("BASSK_PHASE", "9"))
    nc = bacc.Bacc("TRN2", target_bir_lowering=False, debug=False)

    x_d = nc.dram_tensor("x", (S, D), F32, kind="ExternalInput").ap()
    xbf_d = nc.dram_tensor("x_bf", (S, D), BF, kind="ExternalInput").ap()
    wqT_d = nc.dram_tensor("wqT", (D, D), BF, kind="ExternalInput").ap()
    woT_d = nc.dram_tensor("woT", (D, D), BF, kind="ExternalInput").ap()
    w1T_d = nc.dram_tensor("w1T", (D, DFF), BF, kind="ExternalInput").ap()
    w2T_d = nc.dram_tensor("w2T", (DFF, D), BF, kind="ExternalInput").ap()
    bq_d = nc.dram_tensor("bq_v", (P, DC), F32, kind="ExternalInput").ap()
    b1_d = nc.dram_tensor("b1_v", (P, FC), F32, kind="ExternalInput").ap()
    bo_d = nc.dram_tensor("bo_rep", (P, D), BF, kind="ExternalInput").ap()
    b2_d = nc.dram_tensor("b2_rep", (P, D), BF, kind="ExternalInput").ap()
    if not mask_all_ones:
        m01_d = nc.dram_tensor("m01_v", (P, SC), F32, kind="ExternalInput").ap()
    out_d = nc.dram_tensor("out", (S, D), F32, kind="ExternalOutput").ap()

    x_r = x_d.rearrange("(sc p) d -> sc p d", p=P)
    xbf_r = xbf_d.rearrange("(sc p) d -> sc p d", p=P)
    wqT_r = wqT_d.rearrange("(kc p) o -> kc p o", p=P)
    woT_r = woT_d.rearrange("(oc p) d -> oc p d", p=P)
    w1_batched = w1T_d.rearrange("(dc p) f -> p dc f", p=P)
    w2_batched = w2T_d.rearrange("(fc p) d -> p fc d", p=P)
    out_r = out_d.rearrange("(sc p) d -> sc p d", p=P)

    with tile.TileContext(nc) as tc, ExitStack() as st:
        arena = st.enter_context(tc.tile_pool(name="arena", bufs=1))
        small = st.enter_context(tc.tile_pool(name="small", bufs=1))

        # ---- constants ----
        ident_b = small.tile([P, P], BF, name="ident_b")
        make_identity(nc, ident_b[:])
        ones_b = small.tile([P, P], BF, name="ones_b")
        nc.gpsimd.memset(ones_b[:], 1.0)
        ebias = small.tile([P, 1], F32, name="ebias")
        nc.gpsimd.memset(ebias[:], EXP_SHIFT)
        bq_sb = small.tile([P, DC], F32, name="bq_sb")
        b1_sb = small.tile([P, FC], F32, name="b1_sb")
        bo_rep = small.tile([P, D], BF, name="bo_rep")
        b2_rep = small.tile([P, D], BF, name="b2_rep")
        if not mask_all_ones:
            m01_sb = small.tile([P, SC], F32, name="m01_sb")
            nc.sync.dma_start(m01_sb[:], m01_d)

        def emit_bias_dmas():
            # biases are needed late; keep their DMAs behind the x chunks
            nc.sync.dma_start(bq_sb[:], bq_d)
            nc.sync.dma_start(b1_sb[:], b1_d)
            nc.sync.dma_start(bo_rep[:], bo_d)
            nc.sync.dma_start(b2_rep[:], b2_d)

        dma_engines = [nc.sync, nc.scalar, nc.gpsimd]
        # DMA issue queues that never carry softmax exps — weight streams
        # during the attention/FFN overlap must not block the Scalar queue
        # (DMA_DIRECT2D there waits on FFN semaphores ahead of exps).
        dma_quiet = [nc.sync, nc.gpsimd]

        # ---- phase A inputs ----
        # The startup is chip-HBM-contention-bound (all 8 cores load at
        # once), so LN1 reads a bf16 copy of x — half the bytes.  The f32 x
        # is only needed for the residual and is re-loaded later, off the
        # critical path.  GpSimd carries no weight DMAs so the LN applies
        # are not queued behind arrivals.
        xt = arena.tile([P, SC, D], BF, tag="xt_h1", name="xt")
        x_eng = [nc.sync, nc.scalar, nc.gpsimd, nc.sync,
                 nc.scalar, nc.gpsimd, nc.sync, nc.scalar]
        for sc in range(SC):
            x_eng[sc].dma_start(xt[:, sc], xbf_r[sc])
        qq = arena.tile([P, 2 * DC, S], BF, tag="qq_out", name="qq")
        qT = qq[:, 0:DC]        # [o%P, oc, s]
        qh = qq[:, DC:2 * DC]   # [s%P, sc, o]
        n1 = arena.tile([P, SC, D], BF, tag="n1_ctx", name="n1")
        n1T = arena.tile([P, DC, S], BF, tag="n1T_woT", name="n1T")
        wq_sb = arena.tile([P, DC, D], BF, tag="wq_res1", name="wq_sb")
        for kc in range(DC):
            [nc.sync, nc.scalar][kc % 2].dma_start(wq_sb[:, kc], wqT_r[kc])
        emit_bias_dmas()

        # ================= phase A: LN1, q projection, transposes ============
        # LN1 split into halves so the first q-proj matmuls (which only read
        # n1T columns 0:512 = seq chunks 0-3) start while LN of chunks 4-7
        # still runs on the DVE.
        with tc.tile_pool(name="psA", bufs=1, space="PSUM") as psA:
            # HAM warm-up: dead transposes keep the PE clock-gate's busy
            # window active while LN1 runs, so phase A starts at 2.4 GHz
            for _ in range(72):
                wup = psA.tile([P, P], BF, tag="tp", bufs=4, name="wup")
                nc.tensor.transpose(wup[:], ident_b[:], ident_b[:])
            # LN1 in chunk pairs: stats on Vector, applies on GpSimd, and
            # the n1 -> n1T transposes as XBAR DMA-transposes on Sync (one
            # [128,1024] DMA per chunk writes the whole strided slice) —
            # three disjoint queues, nothing blocks the next pair's stats,
            # and the PE does no transpose work at all.
            for pi in range(SC // 2):
                pr = range(2 * pi, 2 * pi + 2)
                _emit_layernorm(nc, small, xt, n1[:, ds(2 * pi, 2)],
                                ln1a, ln1b, f"1p{pi}", pr,
                                apply_eng=nc.gpsimd)
                for ca in pr:
                    nc.sync.dma_start_transpose(
                        n1T[:, :, ts(ca, P)], n1[:, ca])

            def qproj_oc(pool, tag, bufs, b, oc):
                pbt = pool.tile([P, 512], F32, tag=tag, bufs=bufs,
                                name="qps")
                for kc in range(DC):
                    nc.tensor.matmul(
                        pbt[:], wq_sb[:, kc, ts(oc, P)],
                        n1T[:, kc, ds(512 * b, 512)],
                        start=(kc == 0), stop=(kc == DC - 1),
                    )
                nc.vector.tensor_scalar(
                    qT[:, oc, ds(512 * b, 512)], pbt[:],
                    bq_sb[:, ds(oc, 1)], None, OP.add,
                )

            for oc in range(DC):
                qproj_oc(psA, "qps", 4, 0, oc)
                # qh for seq chunks 0-3 only needs qT columns 0:512 (b=0):
                # one XBAR DMA-transpose per oc right after its bias-evac
                nc.sync.dma_start_transpose(
                    qh[:, 0:SC // 2, ts(oc, P)], qT[:, oc, ds(0, 512)])

        def mk_prolog(psB):
            def prolog(hp):
                # deferred phase-A work for feature chunk hp, filling the PE
                # under attention half 0's exp-bound loop: q-proj columns
                # 512:1024 plus one DMA-transpose for the dependent qh half.
                qproj_oc(psB, "mix", 2, 1, hp)
                nc.sync.dma_start_transpose(
                    qh[:, SC // 2:SC, ts(hp, P)], qT[:, hp, ds(512, 512)])
            return prolog

        if phase_stop <= 1:
            for sc in range(SC):
                dt_ = arena.tile([P, D], F32, tag="dump", bufs=2, name="dump")
                nc.vector.tensor_copy(dt_[:], qh[:, sc])
                nc.sync.dma_start(out_r[sc], dt_[:])
            nc.compile()
            return nc

        # persistent across the pipelined halves
        ctxT = arena.tile([P, DC, S], BF, tag="n1_ctx", name="ctxT")
        woT_sb = arena.tile([P, DC, D], BF, tag="n1T_woT", name="woT_sb")
        for oc in range(DC):
            nc.sync.dma_start(woT_sb[:, oc], woT_r[oc])
        res1 = arena.tile([P, SC, D], F32, tag="wq_res1", name="res1")
        out_sb = None

        # ============ attention / wo / LN2 / FFN pipelined by query halves ===
        #
        # exp (ScalarE) is the serial bottleneck of attention; splitting all
        # loops over queries lets FFN(half 0) matmuls run under the exps of
        # attention(half 1).  Z = column sums of E come from ones-matmuls
        # (exact, mask-friendly), replicated across psum partitions so the
        # 1/Z normalization is a plain tensor_tensor against the ctx psum.

        def attn_half(psT, half, prolog=None):
            # Software-pipelined: scores(i+1) matmuls are EMITTED before
            # Z/ctx(i) so the in-order PE queue runs them underneath exp(i).
            # scp is double-buffered (2x2 banks); exp(i-1) must be done
            # reading before scores(i+1) write the same buffer.
            # `prolog(hp)` lets deferred phase-A work (q-proj b1, qh
            # transposes) fill the PE underneath this ACT-bound loop.
            iters = [(hp, c) for hp in range(H // 2) for c in range(SC)]
            state = {}

            def emit_scores(hp, c):
                sp = psT.tile([P, 1024], F32, tag="scp", bufs=2, name="scp")
                for hl in range(2):
                    lo = hl * DK
                    nc.tensor.matmul(
                        sp[:, ds(hl * 512, 512)],
                        qT[ds(lo, DK), hp, ts(c, P)],
                        qT[ds(lo, DK), hp, ds(512 * half, 512)],
                        start=True, stop=True,
                        tile_position=(lo, 0),
                    )
                return sp

            sp_pend = emit_scores(*iters[0])
            for idx, (hp, c) in enumerate(iters):
                sp = sp_pend
                sp_pend = (emit_scores(*iters[idx + 1])
                           if idx + 1 < len(iters) else None)
                if hp not in state:
                    if prolog is not None:
                        prolog(hp)
                    state[hp] = (
                        psT.tile([P, 512], F32, tag="ctxp", bufs=1,
                                 name="ctxp"),
                        psT.tile([P, 512], F32, tag="zps", bufs=1,
                                 name="zps"),
                        arena.tile([P, SC, 2 * 512], BF, tag="EC", bufs=2,
                                   name="ec"),
                    )
                cp, zp, ec = state[hp]
                nc.scalar.activation(
                    ec[:, c], sp[:], AF.Exp, bias=ebias[:], scale=0.125,
                )
                if not mask_all_ones:
                    nc.vector.tensor_scalar_mul(
                        ec[:, c], ec[:, c], m01_sb[:, ds(c, 1)],
                    )
                for hl in range(2):
                    nc.tensor.matmul(
                        zp[ds(hl * DK, DK), :],
                        ones_b[:, ds(hl * DK, DK)],
                        ec[:, c, ds(hl * 512, 512)],
                        start=(c == 0), stop=(c == SC - 1),
                        tile_position=(0, hl * DK),
                        skip_group_check=True,
                    )
                    nc.tensor.matmul(
                        cp[ds(hl * DK, DK), :],
                        qh[:, c, ds(hp * P + hl * DK, DK)],
                        ec[:, c, ds(hl * 512, 512)],
                        start=(c == 0), stop=(c == SC - 1),
                        tile_position=(0, hl * DK),
                        skip_group_check=True,
                    )
                if c == SC - 1:
                    rz = arena.tile([P, 512], F32, tag="rzab", bufs=2,
                                    name="rz")
                    nc.vector.reciprocal_approx_fast(rz[:], zp[:])
                    nc.vector.tensor_mul(
                        ctxT[:, hp, ds(512 * half, 512)], cp[:], rz[:],
                    )
                    del state[hp]

        def wo_half(psW, half):
            for sl in range(SC // 2):
                sc = half * (SC // 2) + sl
                xre = arena.tile([P, D], F32, tag="xre", bufs=2, name="xre")
                nc.sync.dma_start(xre[:], x_r[sc])
                # precombine x + bo off the critical path so each wo PSUM
                # bank frees after a single add
                nc.vector.tensor_add(xre[:], xre[:], bo_rep[:])
                for dh in range(2):
                    wp = psW.tile([P, 512], F32, tag="mix", bufs=2,
                                  name="wops")
                    for oc in range(DC):
                        nc.tensor.matmul(
                            wp[:], ctxT[:, oc, ts(sc, P)],
                            woT_sb[:, oc, ds(512 * dh, 512)],
                            start=(oc == 0), stop=(oc == DC - 1),
                        )
                    nc.vector.tensor_add(
                        res1[:, sc, ds(512 * dh, 512)], wp[:],
                        xre[:, ds(512 * dh, 512)],
                    )

        def ln2_half(psB, half):
            n2h = arena.tile([P, SC // 2, D], BF, tag="n2h", bufs=1,
                             name="n2h")
            chunks = range(half * (SC // 2), (half + 1) * (SC // 2))
            _emit_layernorm(nc, small, res1, n2h, ln2a, ln2b, f"2h{half}",
                            chunks)
            n2Th = arena.tile([P, DC, 512], BF, tag="n2th", bufs=1,
                              name="n2Th")
            for ca in range(SC // 2):
                nc.sync.dma_start_transpose(n2Th[:, :, ts(ca, P)],
                                            n2h[:, ca])
            for sc in chunks:
                nc.vector.tensor_add(res1[:, sc], res1[:, sc], b2_rep[:])
            return n2Th

        def ffn1_half(psB, wsp, half, n2Th):
            h1 = arena.tile([P, FC, 512], BF, tag="xt_h1", name="h1")
            # two queues hide the per-DMA completion latency.  Half 0 runs
            # concurrently with attention-half-1 exps, so its second queue
            # is Sync (Scalar would stall exps behind buffer-gated DMAs);
            # half 1 runs when Scalar is exp-free.
            w1q = [nc.gpsimd, nc.sync if half == 0 else nc.scalar]
            for fc in range(FC):
                wts = wsp.tile([P, DC, P], BF, tag="w1s", bufs=3, name="w1s")
                w1q[fc % 2].dma_start(wts[:], w1_batched[:, :, ts(fc, P)])
                fp = psB.tile([P, 512], F32, tag="mix", bufs=2, name="f1ps")
                for dc in range(DC):
                    nc.tensor.matmul(
                        fp[:], wts[:, dc], n2Th[:, dc, :],
                        start=(dc == 0), stop=(dc == DC - 1),
                    )
                nc.vector.tensor_scalar(
                    h1[:, fc], fp[:], b1_sb[:, ds(fc, 1)], 0.0,
                    OP.add, OP.max,
                )
            return h1

        def ffn2_half(psF2, wsp, half, h1):
            nonlocal out_sb
            if out_sb is None:
                out_sb = arena.tile([P, SC, D], F32, tag="qq_out",
                                    name="out_sb")
            for dh in range(2):
                ops = [psF2.tile([P, 512], F32, tag="f2ps", bufs=6,
                                 name="f2ps") for _ in range(4)]
                for fc2 in range(FC // 2):
                    w2t = wsp.tile([P, 2, 512], BF, tag="w2s", bufs=3,
                                   name="w2s")
                    [nc.gpsimd, nc.scalar][fc2 % 2].dma_start(
                        w2t[:],
                        w2_batched[:, ds(2 * fc2, 2), ds(512 * dh, 512)])
                    for fi in range(2):
                        fc = 2 * fc2 + fi
                        for sl in range(4):
                            nc.tensor.matmul(
                                ops[sl][:], h1[:, fc, ts(sl, P)], w2t[:, fi],
                                start=(fc == 0), stop=(fc == FC - 1),
                            )
                for sl in range(4):
                    sc = half * 4 + sl
                    nc.vector.tensor_add(
                        out_sb[:, sc, ds(512 * dh, 512)], ops[sl][:],
                        res1[:, sc, ds(512 * dh, 512)],
                    )
                    nc.sync.dma_start(
                        out_r[sc][:, ds(512 * dh, 512)],
                        out_sb[:, sc, ds(512 * dh, 512)],
                    )

        with tc.tile_pool(name="psMix", bufs=1, space="PSUM") as psB, \
             tc.tile_pool(name="wstream", bufs=1) as wsp:
            with tc.tile_pool(name="psAtt", bufs=1, space="PSUM") as psT:
                attn_half(psT, 0, prolog=mk_prolog(psB))
                wo_half(psB, 0)
                n2Th0 = ln2_half(psB, 0)
                h10 = ffn1_half(psB, wsp, 0, n2Th0)
                attn_half(psT, 1)
                wo_half(psB, 1)
                n2Th1 = ln2_half(psB, 1)
            with tc.tile_pool(name="psF2", bufs=1, space="PSUM") as psF2:
                ffn2_half(psF2, wsp, 0, h10)
                h11 = ffn1_half(psB, wsp, 1, n2Th1)
                ffn2_half(psF2, wsp, 1, h11)

    nc.compile()
    return nc


def _prep_inputs(inputs):
    f32 = lambda a: np.ascontiguousarray(np.asarray(a, dtype=np.float32))
    bfT = lambda a: np.ascontiguousarray(
        np.asarray(a, dtype=np.float32).T.astype(ml_dtypes.bfloat16))
    x = f32(inputs["x"])                      # [B, S, D]
    mask = np.asarray(inputs["src_mask"])     # [B, 1, 1, S] int32
    wqT = bfT(inputs["wq"])                   # [D, D] (in, out)
    woT = bfT(inputs["wo"])
    w1T = bfT(inputs["w1"])                   # [D, DFF]
    w2T = bfT(inputs["w2"])                   # [DFF, D]
    bq_v = np.ascontiguousarray(f32(inputs["bq"]).reshape(DC, P).T)
    b1_v = np.ascontiguousarray(f32(inputs["b1"]).reshape(FC, P).T)
    bo_rep = np.ascontiguousarray(
        np.tile(f32(inputs["bo"]), (P, 1)).astype(ml_dtypes.bfloat16))
    b2_rep = np.ascontiguousarray(
        np.tile(f32(inputs["b2"]), (P, 1)).astype(ml_dtypes.bfloat16))
    scal = lambda k: float(np.asarray(inputs[k]).reshape(-1)[0])
    ln = (scal("ln1_a"), scal("ln1_b"), scal("ln2_a"), scal("ln2_b"))
    mask_all_ones = bool((mask != 0).all())

    shared = dict(wqT=wqT, woT=woT, w1T=w1T, w2T=w2T, bq_v=bq_v, b1_v=b1_v,
                  bo_rep=bo_rep, b2_rep=b2_rep)
    in_maps = []
    for b in range(NB):
        m = dict(shared)
        m["x"] = np.ascontiguousarray(x[b])
        m["x_bf"] = np.ascontiguousarray(x[b].astype(ml_dtypes.bfloat16))
        if not mask_all_ones:
            m01 = (mask[b].reshape(S) != 0).astype(np.float32)
            m["m01_v"] = np.ascontiguousarray(m01.reshape(SC, P).T)
            m["m01_rep"] = np.ascontiguousarray(np.tile(m01, (P, 1)))
        in_maps.append(m)
    return in_maps, ln, mask_all_ones


last_nc = None
last_in_maps = None


def kernel(**inputs):
    global last_nc, last_in_maps
    in_maps, ln, mask_all_ones = _prep_inputs(inputs)
    nc = build_program(*ln, mask_all_ones)
    last_nc, last_in_maps = nc, in_maps
    res = bass_utils.run_bass_kernel_spmd(
        nc, in_maps, core_ids=list(range(NB)), trace=False,
    )
    out = np.stack([np.asarray(res.results[b]["out"]) for b in range(NB)])
    return out.astype(np.float32)

